# revision 4
# baseline (speedup 1.0000x reference)
"""Self-contained Trainium2 Bass kernel for nn_Encoder (batch-1 LSTM encoder).

Reference computation (H=2048, N=4096):
  xs = [special] + seq + [special]          # [4098, 3]
  x_proj = xs @ W_ih.T + (b_ih + b_hh)      # [4098, 8192]
  scan LSTM cell over 4098 steps; return (h_final, hs[1:])

Strategy: tensor-parallel over the 4H gate dim across 8 NeuronCores.
Each core owns a 256-wide slice of h (and the 4x256 gate rows feeding it).
Per step, per core:
  PE:   4 gate chunks (order i,g,f,o), each = 1 xp-init matmul (K=1) +
        16 k-matmuls (fp32r, W moving [128,256], h k-tile stationary [128,1])
        accumulating into psum [1,256] (one bank per chunk, parity-doubled).
  ACT:  sigmoid/tanh on psum chunks (overlapped under the matvec).
  DVE:  cell update; h = sig_o * tanh(c); copy h -> hs buffer [128,2].
  PE:   2 fp32 K=1 matmuls reshape h [1,256] -> [128,2] (psum).
  Q7:   remote_dma_broadcast of the [128,2] h-slice to all 8 cores
        (register column offset = 2*core_id), double-buffered by step parity.
x_proj is precomputed on host (0.15% of FLOPs) and streamed from DRAM.
"""

import numpy as np

H = 2048
NSEQ = 4096
T_TOTAL = NSEQ + 2  # 4098
NCORES = 8
NBODY = (T_TOTAL - 2) // 4  # 1024 loop bodies of 4 steps
PAD_T = T_TOTAL + 8
GATE_OF_CHUNK = [0, 2, 1, 3]  # chunk order i,g,f,o -> torch row-block index

_CACHED = {}


def _build_nc(T=T_TOTAL):
    import concourse.bass as bass
    import concourse.mybir as mybir
    from concourse.bacc import Bacc

    F32 = mybir.dt.float32
    F32R = mybir.dt.float32r
    ACTF = mybir.ActivationFunctionType

    nbody = (T - 2) // 4
    pad_t = T + 8
    nc = Bacc(num_devices=NCORES)

    w_in = nc.declare_dram_parameter("w_in", [128, 16384], F32R, isOutput=False)
    xp_in = nc.declare_dram_parameter("xp_in", [4, pad_t * 256], F32R, isOutput=False)
    ones_in = nc.declare_dram_parameter("ones_in", [128, 17], F32R, isOutput=False)
    hs_out = nc.declare_dram_parameter("hs_out", [128, 2 * T], F32, isOutput=True)

    ctx_list = []

    class _N:
        pass

    n = _N()

    def sb(name, shape, dt_):
        cm = nc.sbuf_tensor(name, shape, dt_)
        h = cm.__enter__()
        ctx_list.append(cm)
        return h

    def psumt(name, shape, dt_):
        cm = nc.psum_tensor(name, shape, dt_)
        h = cm.__enter__()
        ctx_list.append(cm)
        return h

    w_sb = sb("w_sb", [128, 16384], F32R)
    hs_sb = sb("hs_sb", [128, 2 * T], F32R)
    h_buf = [sb("h_buf0", [128, 16], F32R), sb("h_buf1", [128, 16], F32R)]
    xp_pro = sb("xp_pro", [128, 512], F32R)
    xp_buf = [sb("xp_b0", [128, 512], F32R), sb("xp_b1", [128, 512], F32R)]
    onesr_sb = sb("onesr", [128, 1], F32R)
    ones_sb = sb("ones", [128, 1], F32)
    sig_i = sb("sig_i", [1, 256], F32)
    tg = sb("tg", [1, 256], F32)
    sf = sb("sf", [1, 256], F32)
    so = sb("so", [1, 256], F32)
    tc = sb("tc", [1, 256], F32)
    t1 = sb("t1", [1, 256], F32)
    c_st = sb("c_st", [1, 256], F32)
    h_flat = [sb("h_flat0", [1, 256], F32), sb("h_flat1", [1, 256], F32)]

    ps = [psumt(f"ps{b}", [128, 512], F32) for b in range(8)]
    psum_h = [ps[0][:, 256:258], ps[4][:, 256:258]]

    s_dma = nc.alloc_semaphore("s_dma")
    s_init = nc.alloc_semaphore("s_init")
    s_xp = nc.alloc_semaphore("s_xp")
    s_pe_chunk = nc.alloc_semaphore("s_pe_chunk")  # +4/step
    s_pe_tr = nc.alloc_semaphore("s_pe_tr")  # +1/step
    s_act = nc.alloc_semaphore("s_act")  # +5/step
    s_dve = nc.alloc_semaphore("s_dve")  # +5/step
    s_r = [nc.alloc_semaphore("s_r0"), nc.alloc_semaphore("s_r1")]  # arrivals
    s_l = [nc.alloc_semaphore("s_l0"), nc.alloc_semaphore("s_l1")]  # local send
    s_prep = nc.alloc_semaphore("s_prep")

    with nc.Block() as block:

        # ---------------- SYNC: loads + xp streaming + final store -----------
        @block.sync
        def _(sync):
            sync.dma_start(out=w_sb[:], in_=w_in[:]).then_inc(s_dma, 16)
            with nc.allow_non_contiguous_dma("one-time 128x4B const load"):
                sync.dma_start(out=onesr_sb[:], in_=ones_in[:, 0:1]).then_inc(s_dma, 16)
            sync.dma_start(out=h_buf[0][:], in_=ones_in[:, 1:17]).then_inc(s_dma, 16)
            # xp prologue: steps 0,1 -> xp_pro; steps 2,3 -> buf0; 4,5 -> buf1
            sync.dma_start(
                out=xp_pro[0:128:32, 0:512], in_=xp_in[0:4, 0:512]
            ).then_inc(s_dma, 16)
            sync.dma_start(
                out=xp_buf[0][0:128:32, 0:512], in_=xp_in[0:4, 512:1024]
            ).then_inc(s_dma, 16)
            sync.dma_start(
                out=xp_buf[1][0:128:32, 0:512], in_=xp_in[0:4, 1024:1536]
            ).then_inc(s_dma, 16)
            r_cons = sync.alloc_register("r_cons")
            sync.reg_mov(r_cons, 24)
            with sync.Fori(0, nbody) as jv:
                # body j loads xp for body j+1; guard: PE consumed the buffers
                # being overwritten (all chunks of steps <= 4j+3 done)
                sync.wait_ge(s_pe_chunk, r_cons)
                sync.reg_add(r_cons, r_cons, 16)
                base = jv * 1024 + 1536
                sync.dma_start(
                    out=xp_buf[0][0:128:32, 0:512],
                    in_=xp_in[0:4, bass.ds(base, 512)],
                ).then_inc(s_xp, 16)
                sync.dma_start(
                    out=xp_buf[1][0:128:32, 0:512],
                    in_=xp_in[0:4, bass.ds(base + 512, 512)],
                ).then_inc(s_xp, 16)
            # final: all steps done -> store hs; drain remote traffic
            sync.wait_ge(s_dve, 5 * T)
            sync.dma_start(out=hs_out[:], in_=hs_sb[:].bitcast(F32)).then_inc(
                s_dma, 16
            )
            sync.wait_ge(s_r[0], 16 * (T // 2))
            sync.wait_ge(s_r[1], 16 * (T // 2))
            sync.wait_ge(s_l[0], 16 * (T // 2))
            sync.wait_ge(s_l[1], 16 * (T // 2))
            sync.wait_ge(s_dma, 112)

        # ---------------- GPSIMD: memsets + broadcast loop -------------------
        @block.gpsimd
        def _(gp):
            gp.memset(c_st[:], 0.0).then_inc(s_init, 1)  # s_init: 2 total
            gp.memset(ones_sb[:], 1.0).then_inc(s_init, 1)
            pid = gp.partition_id()
            mycol = pid * 2

            r_dve = gp.alloc_register("r_dve")
            gp.reg_mov(r_dve, 5)

            def bcast(t_step, col):
                # broadcast hs[:, 2t:2t+2] into h_buf[(t+1)&1] col 2*pid
                dstb = h_buf[(t_step + 1) & 1]
                gp.remote_dma_broadcast(
                    out_ap=dstb[:, bass.ds(mycol, 2)],
                    in_ap=hs_sb[:, col],
                    remote_sem=s_r[(t_step + 1) & 1],
                    local_sem=s_l[t_step & 1],
                    rdests=[(0, k) for k in range(NCORES)],
                ).then_inc(s_prep, 1)

            # prologue steps 0,1
            for t in (0, 1):
                bcast(t, slice(2 * t, 2 * t + 2))
            gp.wait_ge(s_prep, 2)
            gp.wait_ge(s_dve, 5)  # copy of step 0 done
            gp.trigger_dma(1)
            gp.wait_ge(s_dve, 10)
            gp.trigger_dma(1)

            r_bp0 = gp.alloc_register("r_bp0")
            r_bp1 = gp.alloc_register("r_bp1")
            r_prep = gp.alloc_register("r_prep")
            gp.reg_mov(r_bp0, 0)
            gp.reg_mov(r_bp1, 0)
            gp.reg_mov(r_prep, 3)
            gp.reg_mov(r_dve, 15)
            with gp.Fori(0, nbody) as jv:
                # backpressure: keep ~2 bodies of sends in flight per parity
                gp.wait_ge(s_l[0], r_bp0)
                gp.wait_ge(s_l[1], r_bp1)
                gp.reg_add(r_bp0, r_bp0, 32)
                gp.reg_add(r_bp1, r_bp1, 32)
                for k in range(4):
                    t_par = (2 + k) & 1  # parity of t = 4j+2+k
                    bcast(2 + k, bass.ds(jv * 8 + 4 + 2 * k, 2))
                    gp.wait_ge(s_prep, r_prep)
                    gp.reg_add(r_prep, r_prep, 1)
                    gp.wait_ge(s_dve, r_dve)
                    gp.reg_add(r_dve, r_dve, 5)
                    gp.trigger_dma(1)

        # ---------------- PE: matvec + reshape matmuls -----------------------
        @block.tensor
        def _(pe):
            def xp_mms(src_ap_cols, par):
                for jc in range(4):
                    pe.matmul(
                        ps[4 * par + jc][0:1, 0:256],
                        lhsT=onesr_sb[32 * jc : 32 * jc + 1, 0:1],
                        rhs=src_ap_cols(jc),
                        start=True,
                        stop=False,
                        tile_position=(32 * jc, 0),
                    )

            def k_mms(par):
                for jc in range(4):
                    for tau in range(16):
                        ins = pe.matmul(
                            ps[4 * par + jc][0:1, 0:256],
                            lhsT=h_buf[par][:, tau : tau + 1],
                            rhs=w_sb[
                                :, (jc * 16 + tau) * 256 : (jc * 16 + tau + 1) * 256
                            ],
                            start=False,
                            stop=(tau == 15),
                        )
                        if tau == 15:
                            ins.then_inc(s_pe_chunk, 1)

            def reshape_mms(t_par):
                hf = h_flat[t_par]
                pe.matmul(
                    psum_h[t_par][:, 0:1],
                    lhsT=hf[0:1, 0:128],
                    rhs=ones_sb[0:1, 0:1],
                    start=True,
                    stop=True,
                )
                pe.matmul(
                    psum_h[t_par][:, 1:2],
                    lhsT=hf[0:1, 128:256],
                    rhs=ones_sb[0:1, 0:1],
                    start=True,
                    stop=True,
                ).then_inc(s_pe_tr, 1)

            # prologue
            pe.wait_ge(s_dma, 96)
            pe.wait_ge(s_init, 2)
            # t=0: h_buf0 zeros
            xp_mms(lambda jc: xp_pro[32 * jc : 32 * jc + 1, 0:256], 0)
            k_mms(0)
            pe.wait_ge(s_dve, 4)
            reshape_mms(0)
            # t=1
            xp_mms(lambda jc: xp_pro[32 * jc : 32 * jc + 1, 256:512], 1)
            pe.wait_ge(s_r[1], 16)
            k_mms(1)
            pe.wait_ge(s_dve, 9)
            reshape_mms(1)

            r_ps = [pe.alloc_register(f"r_ps{p}") for p in range(2)]
            r_ra = [pe.alloc_register(f"r_ra{p}") for p in range(2)]
            r_hf = [pe.alloc_register(f"r_hf{p}") for p in range(2)]
            r_xp = pe.alloc_register("r_xp")
            pe.reg_mov(r_ps[0], 5)  # 5t-5 at t=2
            pe.reg_mov(r_ps[1], 10)  # t=3
            pe.reg_mov(r_ra[0], 16)  # t=2
            pe.reg_mov(r_ra[1], 32)  # t=3
            pe.reg_mov(r_hf[0], 14)  # 5t+4 at t=2
            pe.reg_mov(r_hf[1], 19)  # t=3
            pe.reg_mov(r_xp, 0)
            with pe.Fori(0, nbody) as jv:
                pe.wait_ge(s_xp, r_xp)
                pe.reg_add(r_xp, r_xp, 32)
                for k in range(4):
                    par = k & 1
                    buf = xp_buf[k >> 1]
                    cols = (slice(0, 256), slice(256, 512))[k & 1]
                    pe.wait_ge(s_dve, r_ps[par])
                    pe.reg_add(r_ps[par], r_ps[par], 10)
                    xp_mms(lambda jc: buf[32 * jc : 32 * jc + 1, cols], par)
                    pe.wait_ge(s_r[par], r_ra[par])
                    pe.reg_add(r_ra[par], r_ra[par], 16)
                    k_mms(par)
                    pe.wait_ge(s_dve, r_hf[par])
                    pe.reg_add(r_hf[par], r_hf[par], 10)
                    reshape_mms(par)

        # ---------------- ACT ------------------------------------------------
        @block.scalar
        def _(act):
            def chain_acts(par, w_pe, w_dve_start, w_dve_tc):
                if w_dve_start is not None:
                    act.wait_ge(s_dve, w_dve_start)
                bank = ps[4 * par : 4 * par + 4]
                for idx, (dst, fn) in enumerate(
                    [
                        (sig_i, ACTF.Sigmoid),
                        (tg, ACTF.Tanh),
                        (sf, ACTF.Sigmoid),
                        (so, ACTF.Sigmoid),
                    ]
                ):
                    act.wait_ge(s_pe_chunk, w_pe(idx))
                    src = bank[[0, 1, 2, 3][idx]][0:1, 0:256]
                    act.activation(dst[:], src, fn).then_inc(s_act, 1)
                act.wait_ge(s_dve, w_dve_tc)
                act.activation(tc[:], c_st[:], ACTF.Tanh).then_inc(s_act, 1)

            # prologue (immediates)
            chain_acts(0, lambda i: 1 + i, None, 3)
            chain_acts(1, lambda i: 5 + i, 5, 8)

            r_pe = act.alloc_register("r_pe")
            r_ds = act.alloc_register("r_ds")
            r_tc = act.alloc_register("r_tc")
            act.reg_mov(r_pe, 9)
            act.reg_mov(r_ds, 10)  # 5t at t=2
            act.reg_mov(r_tc, 13)  # 5t+3 at t=2
            with act.Fori(0, nbody) as jv:
                for k in range(4):
                    par = k & 1
                    act.wait_ge(s_dve, r_ds)
                    act.reg_add(r_ds, r_ds, 5)
                    bank = ps[4 * par : 4 * par + 4]
                    for idx, (dst, fn) in enumerate(
                        [
                            (sig_i, ACTF.Sigmoid),
                            (tg, ACTF.Tanh),
                            (sf, ACTF.Sigmoid),
                            (so, ACTF.Sigmoid),
                        ]
                    ):
                        act.wait_ge(s_pe_chunk, r_pe)
                        act.reg_add(r_pe, r_pe, 1)
                        act.activation(dst[:], bank[idx][0:1, 0:256], fn).then_inc(
                            s_act, 1
                        )
                    act.wait_ge(s_dve, r_tc)
                    act.reg_add(r_tc, r_tc, 5)
                    act.activation(tc[:], c_st[:], ACTF.Tanh).then_inc(s_act, 1)

        # ---------------- DVE ------------------------------------------------
        @block.vector
        def _(dve):
            def chain_dve(t_par, hs_col, w_act, w_self, w_trwar, w_tr):
                dve.wait_ge(s_act, w_act[0])
                dve.tensor_mul(t1[:], sig_i[:], tg[:]).then_inc(s_dve, 1)
                dve.wait_ge(s_act, w_act[1])
                dve.tensor_mul(c_st[:], sf[:], c_st[:]).then_inc(s_dve, 1)
                dve.wait_ge(s_dve, w_self)
                dve.tensor_add(c_st[:], c_st[:], t1[:]).then_inc(s_dve, 1)
                dve.wait_ge(s_act, w_act[2])
                if w_trwar is not None:
                    dve.wait_ge(s_pe_tr, w_trwar)
                dve.tensor_mul(h_flat[t_par][:], so[:], tc[:]).then_inc(s_dve, 1)
                dve.wait_ge(s_pe_tr, w_tr)
                dve.tensor_copy(hs_sb[:, hs_col], psum_h[t_par][:]).then_inc(
                    s_dve, 1
                )

            chain_dve(0, slice(0, 2), (2, 3, 5), 2, None, 1)
            chain_dve(1, slice(2, 4), (7, 8, 10), 7, None, 2)

            r_a = dve.alloc_register("r_a")
            r_s = dve.alloc_register("r_s")
            r_tw = dve.alloc_register("r_tw")
            r_tr = dve.alloc_register("r_tr")
            dve.reg_mov(r_a, 12)  # 5t+2 at t=2
            dve.reg_mov(r_s, 12)  # own 5t+2
            dve.reg_mov(r_tw, 1)  # t-1 at t=2
            dve.reg_mov(r_tr, 3)  # t+1 at t=2
            with dve.Fori(0, nbody) as jv:
                for k in range(4):
                    par = k & 1
                    dve.wait_ge(s_act, r_a)
                    dve.tensor_mul(t1[:], sig_i[:], tg[:]).then_inc(s_dve, 1)
                    dve.reg_add(r_a, r_a, 1)
                    dve.wait_ge(s_act, r_a)
                    dve.tensor_mul(c_st[:], sf[:], c_st[:]).then_inc(s_dve, 1)
                    dve.wait_ge(s_dve, r_s)
                    dve.reg_add(r_s, r_s, 5)
                    dve.tensor_add(c_st[:], c_st[:], t1[:]).then_inc(s_dve, 1)
                    dve.reg_add(r_a, r_a, 2)
                    dve.wait_ge(s_act, r_a)
                    dve.reg_add(r_a, r_a, 2)
                    dve.wait_ge(s_pe_tr, r_tw)
                    dve.reg_add(r_tw, r_tw, 1)
                    dve.tensor_mul(h_flat[par][:], so[:], tc[:]).then_inc(s_dve, 1)
                    dve.wait_ge(s_pe_tr, r_tr)
                    dve.reg_add(r_tr, r_tr, 1)
                    dve.tensor_copy(
                        hs_sb[:, bass.ds(jv * 8 + 4 + 2 * k, 2)], psum_h[par][:]
                    ).then_inc(s_dve, 1)

    for cm in reversed(ctx_list):
        cm.__exit__(None, None, None)
    nc.finalize()
    return nc


def _prep_inputs(seq, W_ih, W_hh, b_ih, b_hh):
    T = len(seq) + 2
    pad_t = T + 8
    seq = np.asarray(seq, np.float32)
    W_ih = np.asarray(W_ih, np.float32)
    W_hh = np.asarray(W_hh, np.float32)
    b_ih = np.asarray(b_ih, np.float32)
    b_hh = np.asarray(b_hh, np.float32)

    special = np.array([[0.0, 0.0, 1.0]], np.float32)
    xs = np.concatenate([special, seq, special], axis=0)  # [T, 3]
    xp_full = (
        xs.astype(np.float64) @ W_ih.astype(np.float64).T
        + (b_ih.astype(np.float64) + b_hh.astype(np.float64))
    ).astype(np.float32)  # [T, 8192]

    # W_hh viewed [gate(4), core(8), n(256), d(8), cc(2), kappa(128)]
    Wv = W_hh.reshape(4, 8, 256, 8, 2, 128)
    ones = np.zeros((128, 17), np.float32)
    ones[:, 0] = 1.0

    in_maps = []
    for r in range(NCORES):
        w_parts = []
        for jc in range(4):
            blk = Wv[GATE_OF_CHUNK[jc], r]  # [256 n, 8 d, 2 cc, 128 kappa]
            w_parts.append(
                np.ascontiguousarray(
                    blk.transpose(3, 1, 2, 0).reshape(128, 16 * 256)
                )
            )
        w_moving = np.concatenate(w_parts, axis=1)  # [128, 16384]

        xp_core = np.zeros((4, pad_t * 256), np.float32)
        for jc in range(4):
            g = GATE_OF_CHUNK[jc]
            xp_core[jc, : T * 256] = xp_full[
                :, g * 2048 + r * 256 : g * 2048 + (r + 1) * 256
            ].reshape(-1)

        in_maps.append({"w_in": w_moving, "xp_in": xp_core, "ones_in": ones})
    return in_maps


def _assemble(results, T=T_TOTAL):
    full = np.zeros((T, H), np.float32)
    for r in range(NCORES):
        hs_r = results[r]["hs_out"]  # [128, 2T] f32
        a = hs_r.reshape(128, T, 2).transpose(1, 2, 0).reshape(T, 256)
        full[:, r * 256 : (r + 1) * 256] = a
    hs = full[1:]
    h = full[-1]
    return h.copy(), hs.copy()


def kernel(seq, W_ih, W_hh, b_ih, b_hh):
    from concourse.bass_utils import run_bass_kernel_spmd

    if "nc" not in _CACHED:
        _CACHED["nc"] = _build_nc()
    nc = _CACHED["nc"]
    in_maps = _prep_inputs(seq, W_ih, W_hh, b_ih, b_hh)
    res = run_bass_kernel_spmd(nc, in_maps, core_ids=list(range(NCORES)))
    return _assemble(res.results)


# revision 9
# speedup vs baseline: 1.0949x; 1.0949x over previous
"""Self-contained Trainium2 Bass kernel for nn_Encoder (batch-1 LSTM encoder).

Reference computation (H=2048, N=4096):
  xs = [special] + seq + [special]          # [4098, 3]
  x_proj = xs @ W_ih.T + (b_ih + b_hh)      # [4098, 8192]
  scan LSTM cell over 4098 steps; return (h_final, hs[1:])

Strategy: tensor-parallel over the 4H gate dim across 8 NeuronCores.
Each core owns a 256-wide slice of h (and the 4x256 gate rows feeding it).
Per step, per core:
  PE:   4 gate chunks (order i,g,f,o), each = 1 xp-init matmul (K=1) +
        16 k-matmuls (fp32r, W moving [128,256], h k-tile stationary [128,1])
        accumulating into psum [1,256] (one bank per chunk, parity-doubled).
  ACT:  sigmoid/tanh on psum chunks (overlapped under the matvec).
  DVE:  cell update; h = sig_o * tanh(c); copy h -> hs buffer [128,2].
  PE:   2 fp32 K=1 matmuls reshape h [1,256] -> [128,2] (psum).
  Q7:   remote_dma_broadcast of the [128,2] h-slice to all 8 cores
        (register column offset = 2*core_id), double-buffered by step parity.
x_proj is precomputed on host (0.15% of FLOPs) and streamed from DRAM.
"""

import numpy as np

H = 2048
NSEQ = 4096
T_TOTAL = NSEQ + 2  # 4098
NCORES = 8
NBODY = (T_TOTAL - 2) // 4  # 1024 loop bodies of 4 steps
PAD_T = T_TOTAL + 8
GATE_OF_CHUNK = [0, 2, 1, 3]  # chunk order i,g,f,o -> torch row-block index

_CACHED = {}


def _build_nc(T=T_TOTAL, bcast=True, mini=0):
    import concourse.bass as bass
    import concourse.mybir as mybir
    from concourse.bacc import Bacc

    F32 = mybir.dt.float32
    F32R = mybir.dt.float32r
    ACTF = mybir.ActivationFunctionType

    nbody = (T - 2) // 4
    pad_t = T + 8
    nc = Bacc(num_devices=NCORES)

    w_in = nc.declare_dram_parameter("w_in", [128, 16384], F32R, isOutput=False)
    xp_in = nc.declare_dram_parameter("xp_in", [4, pad_t * 256], F32R, isOutput=False)
    ones_in = nc.declare_dram_parameter("ones_in", [128, 17], F32R, isOutput=False)
    hs_out = nc.declare_dram_parameter("hs_out", [128, 2 * T], F32, isOutput=True)

    ctx_list = []

    class _N:
        pass

    n = _N()

    def sb(name, shape, dt_):
        cm = nc.sbuf_tensor(name, shape, dt_)
        h = cm.__enter__()
        ctx_list.append(cm)
        return h

    def psumt(name, shape, dt_):
        cm = nc.psum_tensor(name, shape, dt_)
        h = cm.__enter__()
        ctx_list.append(cm)
        return h

    w_sb = sb("w_sb", [128, 16384], F32R)
    hs_sb = sb("hs_sb", [128, 2 * T], F32R)
    h_buf = [sb("h_buf0", [128, 16], F32R), sb("h_buf1", [128, 16], F32R)]
    xp_pro = sb("xp_pro", [128, 512], F32R)
    xp_buf = [sb("xp_b0", [128, 512], F32R), sb("xp_b1", [128, 512], F32R)]
    onesr_sb = sb("onesr", [128, 1], F32R)
    ones_sb = sb("ones", [128, 1], F32)
    sig_i = sb("sig_i", [1, 256], F32)
    tg = sb("tg", [1, 256], F32)
    sf = sb("sf", [1, 256], F32)
    so = sb("so", [1, 256], F32)
    tc = sb("tc", [1, 256], F32)
    t1 = sb("t1", [1, 256], F32)
    c_st = sb("c_st", [1, 256], F32)
    h_flat = [sb("h_flat0", [1, 256], F32), sb("h_flat1", [1, 256], F32)]

    ps = [psumt(f"ps{b}", [128, 512], F32) for b in range(8)]
    psum_h = [ps[0][:, 256:258], ps[4][:, 256:258]]

    s_dma = nc.alloc_semaphore("s_dma")
    s_init = nc.alloc_semaphore("s_init")
    s_xp = nc.alloc_semaphore("s_xp")
    s_pe_chunk = nc.alloc_semaphore("s_pe_chunk")  # +4/step
    s_pe_tr = nc.alloc_semaphore("s_pe_tr")  # +1/step
    s_act = nc.alloc_semaphore("s_act")  # +5/step
    s_dve = nc.alloc_semaphore("s_dve")  # +5/step
    s_r = [nc.alloc_semaphore("s_r0"), nc.alloc_semaphore("s_r1")]  # arrivals
    s_l = [nc.alloc_semaphore("s_l0"), nc.alloc_semaphore("s_l1")]  # local send
    s_prep = nc.alloc_semaphore("s_prep")

    with nc.Block() as block:

        # ---------------- SYNC: loads + xp streaming + final store -----------
        @block.sync
        def _(sync):
            sync.dma_start(out=w_sb[:], in_=w_in[:]).then_inc(s_dma, 16)
            with nc.allow_non_contiguous_dma("one-time 128x4B const load"):
                sync.dma_start(out=onesr_sb[:], in_=ones_in[:, 0:1]).then_inc(s_dma, 16)
            sync.dma_start(out=h_buf[0][:], in_=ones_in[:, 1:17]).then_inc(s_dma, 16)
            # xp prologue: steps 0,1 -> xp_pro; steps 2,3 -> buf0; 4,5 -> buf1
            sync.dma_start(
                out=xp_pro[0:128:32, 0:512], in_=xp_in[0:4, 0:512]
            ).then_inc(s_dma, 16)
            sync.dma_start(
                out=xp_buf[0][0:128:32, 0:512], in_=xp_in[0:4, 512:1024]
            ).then_inc(s_dma, 16)
            sync.dma_start(
                out=xp_buf[1][0:128:32, 0:512], in_=xp_in[0:4, 1024:1536]
            ).then_inc(s_dma, 16)
            if not (mini & 1):
                r_cons = sync.alloc_register("r_cons")
                sync.reg_mov(r_cons, 24)
                with sync.Fori(0, nbody) as jv:
                    # body j loads xp for body j+1; guard: PE consumed the
                    # buffers being overwritten
                    sync.wait_ge(s_pe_chunk, r_cons)
                    sync.reg_add(r_cons, r_cons, 16)
                    base = jv * 1024 + 1536
                    sync.dma_start(
                        out=xp_buf[0][0:128:32, 0:512],
                        in_=xp_in[0:4, bass.ds(base, 512)],
                    ).then_inc(s_xp, 16)
                    sync.dma_start(
                        out=xp_buf[1][0:128:32, 0:512],
                        in_=xp_in[0:4, bass.ds(base + 512, 512)],
                    ).then_inc(s_xp, 16)
            # final: all steps done -> store hs; drain remote traffic
            sync.wait_ge(s_dve, 5 * T)
            sync.dma_start(out=hs_out[:], in_=hs_sb[:].bitcast(F32)).then_inc(
                s_dma, 16
            )
            if bcast:
                sync.wait_ge(s_r[0], 16 * (T // 2))
                sync.wait_ge(s_r[1], 16 * (T // 2))
                sync.wait_ge(s_l[0], 16 * (T // 2))
                sync.wait_ge(s_l[1], 16 * (T // 2))
            sync.wait_ge(s_dma, 112)

        # ---------------- GPSIMD: memsets + broadcast loop -------------------
        @block.gpsimd
        def _(gp):
            gp.memset(c_st[:], 0.0).then_inc(s_init, 1)  # s_init: 2 total
            gp.memset(ones_sb[:], 1.0).then_inc(s_init, 1)
            if not bcast:
                return
            pid = gp.partition_id()
            mycol = pid * 2

            r_dve = gp.alloc_register("r_dve")
            gp.reg_mov(r_dve, 5)

            def emit_bcast(t_step, col):
                # broadcast hs[:, 2t:2t+2] into h_buf[(t+1)&1] col 2*pid
                dstb = h_buf[(t_step + 1) & 1]
                gp.remote_dma_broadcast(
                    out_ap=dstb[:, bass.ds(mycol, 2)],
                    in_ap=hs_sb[:, col],
                    remote_sem=s_r[(t_step + 1) & 1],
                    local_sem=s_l[t_step & 1],
                    rdests=[(0, k) for k in range(NCORES)],
                ).then_inc(s_prep, 1)

            # prologue steps 0,1
            for t in (0, 1):
                emit_bcast(t, slice(2 * t, 2 * t + 2))
            gp.wait_ge(s_prep, 2)
            gp.wait_ge(s_dve, 5)  # copy of step 0 done
            gp.trigger_dma(1)
            gp.wait_ge(s_dve, 10)
            gp.trigger_dma(1)

            r_bp0 = gp.alloc_register("r_bp0")
            r_bp1 = gp.alloc_register("r_bp1")
            r_prep = gp.alloc_register("r_prep")
            gp.reg_mov(r_bp0, 0)
            gp.reg_mov(r_bp1, 0)
            gp.reg_mov(r_prep, 3)
            gp.reg_mov(r_dve, 15)
            with gp.Fori(0, nbody) as jv:
                for k in range(4):
                    t_par = (2 + k) & 1  # parity of t = 4j+2+k
                    emit_bcast(2 + k, bass.ds(jv * 8 + 4 + 2 * k, 2))
                    gp.wait_ge(s_prep, r_prep)
                    gp.reg_add(r_prep, r_prep, 1)
                    gp.wait_ge(s_dve, r_dve)
                    gp.reg_add(r_dve, r_dve, 5)
                    gp.trigger_dma(1)

        # ---------------- PE: matvec + reshape matmuls -----------------------
        @block.tensor
        def _(pe):
            def xp_mms(src_ap_cols, par):
                for jc in range(4):
                    pe.matmul(
                        ps[4 * par + jc][0:1, 0:256],
                        lhsT=onesr_sb[32 * jc : 32 * jc + 1, 0:1],
                        rhs=src_ap_cols(jc),
                        start=True,
                        stop=False,
                        tile_position=(32 * jc, 0),
                    )

            def k_mms(par):
                for jc in range(4):
                    for tau in range(16):
                        ins = pe.matmul(
                            ps[4 * par + jc][0:1, 0:256],
                            lhsT=h_buf[par][:, tau : tau + 1],
                            rhs=w_sb[
                                :, (jc * 16 + tau) * 256 : (jc * 16 + tau + 1) * 256
                            ],
                            start=False,
                            stop=(tau == 15),
                        )
                        if tau == 15:
                            ins.then_inc(s_pe_chunk, 1)

            def reshape_mms(t_par):
                hf = h_flat[t_par]
                pe.matmul(
                    psum_h[t_par][:, 0:1],
                    lhsT=hf[0:1, 0:128],
                    rhs=ones_sb[0:1, 0:1],
                    start=True,
                    stop=True,
                )
                pe.matmul(
                    psum_h[t_par][:, 1:2],
                    lhsT=hf[0:1, 128:256],
                    rhs=ones_sb[0:1, 0:1],
                    start=True,
                    stop=True,
                ).then_inc(s_pe_tr, 1)

            # prologue
            pe.wait_ge(s_dma, 96)
            pe.wait_ge(s_init, 2)
            # t=0: h_buf0 zeros
            xp_mms(lambda jc: xp_pro[32 * jc : 32 * jc + 1, 0:256], 0)
            k_mms(0)
            pe.wait_ge(s_dve, 4)
            reshape_mms(0)
            # t=1
            xp_mms(lambda jc: xp_pro[32 * jc : 32 * jc + 1, 256:512], 1)
            if bcast:
                pe.wait_ge(s_r[1], 16)
            k_mms(1)
            pe.wait_ge(s_dve, 9)
            reshape_mms(1)

            r_ps = [pe.alloc_register(f"r_ps{p}") for p in range(2)]
            r_ra = [pe.alloc_register(f"r_ra{p}") for p in range(2)]
            r_hf = [pe.alloc_register(f"r_hf{p}") for p in range(2)]
            r_xp = pe.alloc_register("r_xp")
            pe.reg_mov(r_ps[0], 5)  # 5t-5 at t=2
            pe.reg_mov(r_ps[1], 10)  # t=3
            pe.reg_mov(r_ra[0], 16)  # t=2
            pe.reg_mov(r_ra[1], 32)  # t=3
            pe.reg_mov(r_hf[0], 14)  # 5t+4 at t=2
            pe.reg_mov(r_hf[1], 19)  # t=3
            pe.reg_mov(r_xp, 0)
            with pe.Fori(0, nbody) as jv:
                if not (mini & 1):
                    pe.wait_ge(s_xp, r_xp)
                    pe.reg_add(r_xp, r_xp, 32)
                for k in range(4):
                    par = k & 1
                    buf = xp_buf[k >> 1]
                    cols = (slice(0, 256), slice(256, 512))[k & 1]
                    pe.wait_ge(s_dve, r_ps[par])
                    pe.reg_add(r_ps[par], r_ps[par], 10)
                    xp_mms(lambda jc: buf[32 * jc : 32 * jc + 1, cols], par)
                    if bcast:
                        pe.wait_ge(s_r[par], r_ra[par])
                        pe.reg_add(r_ra[par], r_ra[par], 16)
                    k_mms(par)
                    pe.wait_ge(s_dve, r_hf[par])
                    pe.reg_add(r_hf[par], r_hf[par], 10)
                    reshape_mms(par)

        # ---------------- ACT ------------------------------------------------
        @block.scalar
        def _(act):
            def chain_acts(par, w_pe, w_dve_start, w_dve_tc):
                if w_dve_start is not None:
                    act.wait_ge(s_dve, w_dve_start)
                bank = ps[4 * par : 4 * par + 4]
                for idx, (dst, fn) in enumerate(
                    [
                        (sig_i, ACTF.Sigmoid),
                        (tg, ACTF.Tanh),
                        (sf, ACTF.Sigmoid),
                        (so, ACTF.Sigmoid),
                    ]
                ):
                    act.wait_ge(s_pe_chunk, w_pe(idx))
                    src = bank[[0, 1, 2, 3][idx]][0:1, 0:256]
                    act.activation(dst[:], src, fn).then_inc(s_act, 1)
                act.wait_ge(s_dve, w_dve_tc)
                act.activation(tc[:], c_st[:], ACTF.Tanh).then_inc(s_act, 1)

            # prologue (immediates)
            chain_acts(0, lambda i: 1 + i, None, 3)
            chain_acts(1, lambda i: 5 + i, 5, 8)

            r_pe = act.alloc_register("r_pe")
            r_ds = act.alloc_register("r_ds")
            r_tc = act.alloc_register("r_tc")
            act.reg_mov(r_pe, 9)
            act.reg_mov(r_ds, 10)  # 5t at t=2
            act.reg_mov(r_tc, 13)  # 5t+3 at t=2
            with act.Fori(0, nbody) as jv:
                for k in range(4):
                    par = k & 1
                    act.wait_ge(s_dve, r_ds)
                    act.reg_add(r_ds, r_ds, 5)
                    bank = ps[4 * par : 4 * par + 4]
                    for idx, (dst, fn) in enumerate(
                        [
                            (sig_i, ACTF.Sigmoid),
                            (tg, ACTF.Tanh),
                            (sf, ACTF.Sigmoid),
                            (so, ACTF.Sigmoid),
                        ]
                    ):
                        act.wait_ge(s_pe_chunk, r_pe)
                        act.reg_add(r_pe, r_pe, 1)
                        act.activation(dst[:], bank[idx][0:1, 0:256], fn).then_inc(
                            s_act, 1
                        )
                    act.wait_ge(s_dve, r_tc)
                    act.reg_add(r_tc, r_tc, 5)
                    act.activation(tc[:], c_st[:], ACTF.Tanh).then_inc(s_act, 1)

        # ---------------- DVE ------------------------------------------------
        @block.vector
        def _(dve):
            def chain_dve(t_par, hs_col, w_act, w_self, w_trwar, w_tr):
                dve.wait_ge(s_act, w_act[0])
                dve.tensor_mul(t1[:], sig_i[:], tg[:]).then_inc(s_dve, 1)
                dve.wait_ge(s_act, w_act[1])
                dve.tensor_mul(c_st[:], sf[:], c_st[:]).then_inc(s_dve, 1)
                dve.wait_ge(s_dve, w_self)
                dve.tensor_add(c_st[:], c_st[:], t1[:]).then_inc(s_dve, 1)
                dve.wait_ge(s_act, w_act[2])
                if w_trwar is not None:
                    dve.wait_ge(s_pe_tr, w_trwar)
                dve.tensor_mul(h_flat[t_par][:], so[:], tc[:]).then_inc(s_dve, 1)
                dve.wait_ge(s_pe_tr, w_tr)
                dve.tensor_copy(hs_sb[:, hs_col], psum_h[t_par][:]).then_inc(
                    s_dve, 1
                )

            chain_dve(0, slice(0, 2), (2, 3, 5), 2, None, 1)
            chain_dve(1, slice(2, 4), (7, 8, 10), 7, None, 2)

            r_a = dve.alloc_register("r_a")
            r_s = dve.alloc_register("r_s")
            r_tw = dve.alloc_register("r_tw")
            r_tr = dve.alloc_register("r_tr")
            dve.reg_mov(r_a, 12)  # 5t+2 at t=2
            dve.reg_mov(r_s, 12)  # own 5t+2
            dve.reg_mov(r_tw, 1)  # t-1 at t=2
            dve.reg_mov(r_tr, 3)  # t+1 at t=2
            with dve.Fori(0, nbody) as jv:
                for k in range(4):
                    par = k & 1
                    dve.wait_ge(s_act, r_a)
                    dve.tensor_mul(t1[:], sig_i[:], tg[:]).then_inc(s_dve, 1)
                    dve.reg_add(r_a, r_a, 1)
                    dve.wait_ge(s_act, r_a)
                    dve.tensor_mul(c_st[:], sf[:], c_st[:]).then_inc(s_dve, 1)
                    dve.wait_ge(s_dve, r_s)
                    dve.reg_add(r_s, r_s, 5)
                    dve.tensor_add(c_st[:], c_st[:], t1[:]).then_inc(s_dve, 1)
                    dve.reg_add(r_a, r_a, 2)
                    dve.wait_ge(s_act, r_a)
                    dve.reg_add(r_a, r_a, 2)
                    dve.wait_ge(s_pe_tr, r_tw)
                    dve.reg_add(r_tw, r_tw, 1)
                    dve.tensor_mul(h_flat[par][:], so[:], tc[:]).then_inc(s_dve, 1)
                    dve.wait_ge(s_pe_tr, r_tr)
                    dve.reg_add(r_tr, r_tr, 1)
                    dve.tensor_copy(
                        hs_sb[:, bass.ds(jv * 8 + 4 + 2 * k, 2)], psum_h[par][:]
                    ).then_inc(s_dve, 1)

    for cm in reversed(ctx_list):
        cm.__exit__(None, None, None)
    nc.finalize()
    return nc


def _prep_inputs(seq, W_ih, W_hh, b_ih, b_hh):
    T = len(seq) + 2
    pad_t = T + 8
    seq = np.asarray(seq, np.float32)
    W_ih = np.asarray(W_ih, np.float32)
    W_hh = np.asarray(W_hh, np.float32)
    b_ih = np.asarray(b_ih, np.float32)
    b_hh = np.asarray(b_hh, np.float32)

    special = np.array([[0.0, 0.0, 1.0]], np.float32)
    xs = np.concatenate([special, seq, special], axis=0)  # [T, 3]
    xp_full = (
        xs.astype(np.float64) @ W_ih.astype(np.float64).T
        + (b_ih.astype(np.float64) + b_hh.astype(np.float64))
    ).astype(np.float32)  # [T, 8192]

    # W_hh viewed [gate(4), core(8), n(256), d(8), cc(2), kappa(128)]
    Wv = W_hh.reshape(4, 8, 256, 8, 2, 128)
    ones = np.zeros((128, 17), np.float32)
    ones[:, 0] = 1.0

    in_maps = []
    for r in range(NCORES):
        w_parts = []
        for jc in range(4):
            blk = Wv[GATE_OF_CHUNK[jc], r]  # [256 n, 8 d, 2 cc, 128 kappa]
            w_parts.append(
                np.ascontiguousarray(
                    blk.transpose(3, 1, 2, 0).reshape(128, 16 * 256)
                )
            )
        w_moving = np.concatenate(w_parts, axis=1)  # [128, 16384]

        xp_core = np.zeros((4, pad_t * 256), np.float32)
        for jc in range(4):
            g = GATE_OF_CHUNK[jc]
            xp_core[jc, : T * 256] = xp_full[
                :, g * 2048 + r * 256 : g * 2048 + (r + 1) * 256
            ].reshape(-1)

        in_maps.append({"w_in": w_moving, "xp_in": xp_core, "ones_in": ones})
    return in_maps


def _assemble(results, T=T_TOTAL):
    full = np.zeros((T, H), np.float32)
    for r in range(NCORES):
        hs_r = results[r]["hs_out"]  # [128, 2T] f32
        a = hs_r.reshape(128, T, 2).transpose(1, 2, 0).reshape(T, 256)
        full[:, r * 256 : (r + 1) * 256] = a
    hs = full[1:]
    h = full[-1]
    return h.copy(), hs.copy()


def kernel(seq, W_ih, W_hh, b_ih, b_hh):
    from concourse.bass_utils import run_bass_kernel_spmd

    if "nc" not in _CACHED:
        _CACHED["nc"] = _build_nc()
    nc = _CACHED["nc"]
    in_maps = _prep_inputs(seq, W_ih, W_hh, b_ih, b_hh)
    res = run_bass_kernel_spmd(nc, in_maps, core_ids=list(range(NCORES)))
    return _assemble(res.results)


# revision 10
# speedup vs baseline: 1.2035x; 1.0991x over previous
"""Self-contained Trainium2 Bass kernel for nn_Encoder (batch-1 LSTM encoder).

Reference computation (H=2048, N=4096):
  xs = [special] + seq + [special]          # [4098, 3]
  x_proj = xs @ W_ih.T + (b_ih + b_hh)      # [4098, 8192]
  scan LSTM cell over 4098 steps; return (h_final, hs[1:])

Strategy: tensor-parallel over the 4H gate dim across 8 NeuronCores.
Each core owns a 256-wide slice of h (and the 4x256 gate rows feeding it).
Per step, per core:
  PE:   4 gate chunks (order i,g,f,o), each = 1 xp-init matmul (K=1) +
        16 k-matmuls (fp32r, W moving [128,256], h k-tile stationary [128,1])
        accumulating into psum [1,256] (one bank per chunk, parity-doubled).
  ACT:  sigmoid/tanh on psum chunks (overlapped under the matvec).
  DVE:  cell update; h = sig_o * tanh(c); copy h -> hs buffer [128,2].
  PE:   2 fp32 K=1 matmuls reshape h [1,256] -> [128,2] (psum).
  Q7:   remote_dma_broadcast of the [128,2] h-slice to all 8 cores
        (register column offset = 2*core_id), double-buffered by step parity.
x_proj is precomputed on host (0.15% of FLOPs) and streamed from DRAM.
"""

import numpy as np

H = 2048
NSEQ = 4096
T_TOTAL = NSEQ + 2  # 4098
NCORES = 8
NBODY = (T_TOTAL - 2) // 4  # 1024 loop bodies of 4 steps
PAD_T = T_TOTAL + 8
GATE_OF_CHUNK = [0, 2, 1, 3]  # chunk order i,g,f,o -> torch row-block index

_CACHED = {}


def _build_nc(T=T_TOTAL, bcast=True, mini=0):
    import concourse.bass as bass
    import concourse.mybir as mybir
    from concourse.bacc import Bacc

    F32 = mybir.dt.float32
    F32R = mybir.dt.float32r
    F16 = mybir.dt.float16
    ACTF = mybir.ActivationFunctionType

    nbody = (T - 2) // 4
    pad_t = T + 8
    nc = Bacc(num_devices=NCORES)

    w_in = nc.declare_dram_parameter("w_in", [128, 16384], F32R, isOutput=False)
    xp_in = nc.declare_dram_parameter("xp_in", [4, pad_t * 256], F16, isOutput=False)
    ones_in = nc.declare_dram_parameter("ones_in", [128, 17], F32R, isOutput=False)
    hs_out = nc.declare_dram_parameter("hs_out", [128, 2 * T], F32, isOutput=True)

    ctx_list = []

    class _N:
        pass

    n = _N()

    def sb(name, shape, dt_):
        cm = nc.sbuf_tensor(name, shape, dt_)
        h = cm.__enter__()
        ctx_list.append(cm)
        return h

    def psumt(name, shape, dt_):
        cm = nc.psum_tensor(name, shape, dt_)
        h = cm.__enter__()
        ctx_list.append(cm)
        return h

    w_sb = sb("w_sb", [128, 16384], F32R)
    hs_sb = sb("hs_sb", [128, 2 * T], F32R)
    h_buf = [sb("h_buf0", [128, 16], F32R), sb("h_buf1", [128, 16], F32R)]
    xp_pro = sb("xp_pro", [128, 512], F16)
    xp_buf = [sb("xp_b0", [128, 512], F16), sb("xp_b1", [128, 512], F16)]
    onesr_sb = sb("onesr", [128, 1], F32R)
    ones_sb = sb("ones", [128, 1], F32)
    ones16 = sb("ones16", [128, 1], F16)
    sig_i = sb("sig_i", [1, 256], F32)
    tg = sb("tg", [1, 256], F32)
    sf = sb("sf", [1, 256], F32)
    so = sb("so", [1, 256], F32)
    tc = sb("tc", [1, 256], F32)
    t1 = sb("t1", [1, 256], F32)
    c_st = sb("c_st", [1, 256], F32)
    h_flat = [sb("h_flat0", [1, 256], F32), sb("h_flat1", [1, 256], F32)]

    ps = [psumt(f"ps{b}", [128, 512], F32) for b in range(8)]
    psum_h = [ps[0][:, 256:258], ps[4][:, 256:258]]

    s_dma = nc.alloc_semaphore("s_dma")
    s_init = nc.alloc_semaphore("s_init")
    s_xp = nc.alloc_semaphore("s_xp")
    s_pe_chunk = nc.alloc_semaphore("s_pe_chunk")  # +4/step
    s_pe_tr = nc.alloc_semaphore("s_pe_tr")  # +1/step
    s_act = nc.alloc_semaphore("s_act")  # +5/step
    s_dve = nc.alloc_semaphore("s_dve")  # +5/step
    s_r = [nc.alloc_semaphore("s_r0"), nc.alloc_semaphore("s_r1")]  # arrivals
    s_l = [nc.alloc_semaphore("s_l0"), nc.alloc_semaphore("s_l1")]  # local send
    s_prep = nc.alloc_semaphore("s_prep")

    with nc.Block() as block:

        # ---------------- SYNC: loads + xp streaming + final store -----------
        @block.sync
        def _(sync):
            sync.dma_start(out=w_sb[:], in_=w_in[:]).then_inc(s_dma, 16)
            with nc.allow_non_contiguous_dma("one-time 128x4B const load"):
                sync.dma_start(out=onesr_sb[:], in_=ones_in[:, 0:1]).then_inc(s_dma, 16)
            sync.dma_start(out=h_buf[0][:], in_=ones_in[:, 1:17]).then_inc(s_dma, 16)
            # xp prologue: steps 0,1 -> xp_pro; steps 2,3 -> buf0; 4,5 -> buf1
            sync.dma_start(
                out=xp_pro[0:128:32, 0:512], in_=xp_in[0:4, 0:512]
            ).then_inc(s_dma, 16)
            sync.dma_start(
                out=xp_buf[0][0:128:32, 0:512], in_=xp_in[0:4, 512:1024]
            ).then_inc(s_dma, 16)
            sync.dma_start(
                out=xp_buf[1][0:128:32, 0:512], in_=xp_in[0:4, 1024:1536]
            ).then_inc(s_dma, 16)
            if not (mini & 1):
                r_cons = sync.alloc_register("r_cons")
                sync.reg_mov(r_cons, 24)
                with sync.Fori(0, nbody) as jv:
                    # body j loads xp for body j+1; guard: PE consumed the
                    # buffers being overwritten
                    sync.wait_ge(s_pe_chunk, r_cons)
                    sync.reg_add(r_cons, r_cons, 16)
                    base = jv * 1024 + 1536
                    sync.dma_start(
                        out=xp_buf[0][0:128:32, 0:512],
                        in_=xp_in[0:4, bass.ds(base, 512)],
                    ).then_inc(s_xp, 16)
                    sync.dma_start(
                        out=xp_buf[1][0:128:32, 0:512],
                        in_=xp_in[0:4, bass.ds(base + 512, 512)],
                    ).then_inc(s_xp, 16)
            # final: all steps done -> store hs; drain remote traffic
            sync.wait_ge(s_dve, 5 * T)
            sync.dma_start(out=hs_out[:], in_=hs_sb[:].bitcast(F32)).then_inc(
                s_dma, 16
            )
            if bcast:
                sync.wait_ge(s_r[0], 16 * (T // 2))
                sync.wait_ge(s_r[1], 16 * (T // 2))
                sync.wait_ge(s_l[0], 16 * (T // 2))
                sync.wait_ge(s_l[1], 16 * (T // 2))
            sync.wait_ge(s_dma, 112)

        # ---------------- GPSIMD: memsets + broadcast loop -------------------
        @block.gpsimd
        def _(gp):
            gp.memset(c_st[:], 0.0).then_inc(s_init, 1)  # s_init: 2 total
            gp.memset(ones_sb[:], 1.0).then_inc(s_init, 1)
            gp.memset(ones16[:], 1.0).then_inc(s_init, 1)  # 3 total
            if not bcast:
                return
            pid = gp.partition_id()
            mycol = pid * 2

            r_dve = gp.alloc_register("r_dve")
            gp.reg_mov(r_dve, 5)

            def emit_bcast(t_step, col):
                # broadcast hs[:, 2t:2t+2] into h_buf[(t+1)&1] col 2*pid
                dstb = h_buf[(t_step + 1) & 1]
                gp.remote_dma_broadcast(
                    out_ap=dstb[:, bass.ds(mycol, 2)],
                    in_ap=hs_sb[:, col],
                    remote_sem=s_r[(t_step + 1) & 1],
                    local_sem=s_l[t_step & 1],
                    rdests=[(0, k) for k in range(NCORES)],
                ).then_inc(s_prep, 1)

            # prologue steps 0,1
            for t in (0, 1):
                emit_bcast(t, slice(2 * t, 2 * t + 2))
            gp.wait_ge(s_prep, 2)
            gp.wait_ge(s_dve, 5)  # copy of step 0 done
            gp.trigger_dma(1)
            gp.wait_ge(s_dve, 10)
            gp.trigger_dma(1)

            r_bp0 = gp.alloc_register("r_bp0")
            r_bp1 = gp.alloc_register("r_bp1")
            r_prep = gp.alloc_register("r_prep")
            gp.reg_mov(r_bp0, 0)
            gp.reg_mov(r_bp1, 0)
            gp.reg_mov(r_prep, 3)
            gp.reg_mov(r_dve, 15)
            with gp.Fori(0, nbody) as jv:
                for k in range(4):
                    t_par = (2 + k) & 1  # parity of t = 4j+2+k
                    emit_bcast(2 + k, bass.ds(jv * 8 + 4 + 2 * k, 2))
                    gp.wait_ge(s_prep, r_prep)
                    gp.reg_add(r_prep, r_prep, 1)
                    gp.wait_ge(s_dve, r_dve)
                    gp.reg_add(r_dve, r_dve, 5)
                    gp.trigger_dma(1)

        # ---------------- PE: matvec + reshape matmuls -----------------------
        @block.tensor
        def _(pe):
            def xp_mms(src_ap_cols, par):
                for jc in range(4):
                    pe.matmul(
                        ps[4 * par + jc][0:1, 0:256],
                        lhsT=ones16[32 * jc : 32 * jc + 1, 0:1],
                        rhs=src_ap_cols(jc),
                        start=True,
                        stop=False,
                        tile_position=(32 * jc, 0),
                    )

            def k_mms(par):
                for jc in range(4):
                    for tau in range(16):
                        ins = pe.matmul(
                            ps[4 * par + jc][0:1, 0:256],
                            lhsT=h_buf[par][:, tau : tau + 1],
                            rhs=w_sb[
                                :, (jc * 16 + tau) * 256 : (jc * 16 + tau + 1) * 256
                            ],
                            start=False,
                            stop=(tau == 15),
                        )
                        if tau == 15:
                            ins.then_inc(s_pe_chunk, 1)

            def reshape_mms(t_par):
                hf = h_flat[t_par]
                pe.matmul(
                    psum_h[t_par][:, 0:1],
                    lhsT=hf[0:1, 0:128],
                    rhs=ones_sb[0:1, 0:1],
                    start=True,
                    stop=True,
                )
                pe.matmul(
                    psum_h[t_par][:, 1:2],
                    lhsT=hf[0:1, 128:256],
                    rhs=ones_sb[0:1, 0:1],
                    start=True,
                    stop=True,
                ).then_inc(s_pe_tr, 1)

            # prologue
            pe.wait_ge(s_dma, 96)
            pe.wait_ge(s_init, 3)
            # t=0: h_buf0 zeros
            xp_mms(lambda jc: xp_pro[32 * jc : 32 * jc + 1, 0:256], 0)
            k_mms(0)
            pe.wait_ge(s_dve, 4)
            reshape_mms(0)
            # t=1
            xp_mms(lambda jc: xp_pro[32 * jc : 32 * jc + 1, 256:512], 1)
            if bcast:
                pe.wait_ge(s_r[1], 16)
            k_mms(1)
            pe.wait_ge(s_dve, 9)
            reshape_mms(1)

            r_ps = [pe.alloc_register(f"r_ps{p}") for p in range(2)]
            r_ra = [pe.alloc_register(f"r_ra{p}") for p in range(2)]
            r_hf = [pe.alloc_register(f"r_hf{p}") for p in range(2)]
            r_xp = pe.alloc_register("r_xp")
            pe.reg_mov(r_ps[0], 5)  # 5t-5 at t=2
            pe.reg_mov(r_ps[1], 10)  # t=3
            pe.reg_mov(r_ra[0], 16)  # t=2
            pe.reg_mov(r_ra[1], 32)  # t=3
            pe.reg_mov(r_hf[0], 14)  # 5t+4 at t=2
            pe.reg_mov(r_hf[1], 19)  # t=3
            pe.reg_mov(r_xp, 0)
            with pe.Fori(0, nbody) as jv:
                if not (mini & 1):
                    pe.wait_ge(s_xp, r_xp)
                    pe.reg_add(r_xp, r_xp, 32)
                for k in range(4):
                    par = k & 1
                    buf = xp_buf[k >> 1]
                    cols = (slice(0, 256), slice(256, 512))[k & 1]
                    pe.wait_ge(s_dve, r_ps[par])
                    pe.reg_add(r_ps[par], r_ps[par], 10)
                    xp_mms(lambda jc: buf[32 * jc : 32 * jc + 1, cols], par)
                    if bcast:
                        pe.wait_ge(s_r[par], r_ra[par])
                        pe.reg_add(r_ra[par], r_ra[par], 16)
                    k_mms(par)
                    pe.wait_ge(s_dve, r_hf[par])
                    pe.reg_add(r_hf[par], r_hf[par], 10)
                    reshape_mms(par)

        # ---------------- ACT ------------------------------------------------
        @block.scalar
        def _(act):
            def chain_acts(par, w_pe, w_dve_start, w_dve_tc):
                if w_dve_start is not None:
                    act.wait_ge(s_dve, w_dve_start)
                bank = ps[4 * par : 4 * par + 4]
                for idx, (dst, fn) in enumerate(
                    [
                        (sig_i, ACTF.Sigmoid),
                        (tg, ACTF.Tanh),
                        (sf, ACTF.Sigmoid),
                        (so, ACTF.Sigmoid),
                    ]
                ):
                    act.wait_ge(s_pe_chunk, w_pe(idx))
                    src = bank[[0, 1, 2, 3][idx]][0:1, 0:256]
                    act.activation(dst[:], src, fn).then_inc(s_act, 1)
                act.wait_ge(s_dve, w_dve_tc)
                act.activation(tc[:], c_st[:], ACTF.Tanh).then_inc(s_act, 1)

            # prologue (immediates)
            chain_acts(0, lambda i: 1 + i, None, 3)
            chain_acts(1, lambda i: 5 + i, 5, 8)

            r_pe = act.alloc_register("r_pe")
            r_ds = act.alloc_register("r_ds")
            r_tc = act.alloc_register("r_tc")
            act.reg_mov(r_pe, 9)
            act.reg_mov(r_ds, 10)  # 5t at t=2
            act.reg_mov(r_tc, 13)  # 5t+3 at t=2
            with act.Fori(0, nbody) as jv:
                for k in range(4):
                    par = k & 1
                    act.wait_ge(s_dve, r_ds)
                    act.reg_add(r_ds, r_ds, 5)
                    bank = ps[4 * par : 4 * par + 4]
                    for idx, (dst, fn) in enumerate(
                        [
                            (sig_i, ACTF.Sigmoid),
                            (tg, ACTF.Tanh),
                            (sf, ACTF.Sigmoid),
                            (so, ACTF.Sigmoid),
                        ]
                    ):
                        act.wait_ge(s_pe_chunk, r_pe)
                        act.reg_add(r_pe, r_pe, 1)
                        act.activation(dst[:], bank[idx][0:1, 0:256], fn).then_inc(
                            s_act, 1
                        )
                    act.wait_ge(s_dve, r_tc)
                    act.reg_add(r_tc, r_tc, 5)
                    act.activation(tc[:], c_st[:], ACTF.Tanh).then_inc(s_act, 1)

        # ---------------- DVE ------------------------------------------------
        @block.vector
        def _(dve):
            def chain_dve(t_par, hs_col, w_act, w_self, w_trwar, w_tr):
                dve.wait_ge(s_act, w_act[0])
                dve.tensor_mul(t1[:], sig_i[:], tg[:]).then_inc(s_dve, 1)
                dve.wait_ge(s_act, w_act[1])
                dve.tensor_mul(c_st[:], sf[:], c_st[:]).then_inc(s_dve, 1)
                dve.wait_ge(s_dve, w_self)
                dve.tensor_add(c_st[:], c_st[:], t1[:]).then_inc(s_dve, 1)
                dve.wait_ge(s_act, w_act[2])
                if w_trwar is not None:
                    dve.wait_ge(s_pe_tr, w_trwar)
                dve.tensor_mul(h_flat[t_par][:], so[:], tc[:]).then_inc(s_dve, 1)
                dve.wait_ge(s_pe_tr, w_tr)
                dve.tensor_copy(hs_sb[:, hs_col], psum_h[t_par][:]).then_inc(
                    s_dve, 1
                )

            chain_dve(0, slice(0, 2), (2, 3, 5), 2, None, 1)
            chain_dve(1, slice(2, 4), (7, 8, 10), 7, None, 2)

            r_a = dve.alloc_register("r_a")
            r_s = dve.alloc_register("r_s")
            r_tw = dve.alloc_register("r_tw")
            r_tr = dve.alloc_register("r_tr")
            dve.reg_mov(r_a, 12)  # 5t+2 at t=2
            dve.reg_mov(r_s, 12)  # own 5t+2
            dve.reg_mov(r_tw, 1)  # t-1 at t=2
            dve.reg_mov(r_tr, 3)  # t+1 at t=2
            with dve.Fori(0, nbody) as jv:
                for k in range(4):
                    par = k & 1
                    dve.wait_ge(s_act, r_a)
                    dve.tensor_mul(t1[:], sig_i[:], tg[:]).then_inc(s_dve, 1)
                    dve.reg_add(r_a, r_a, 1)
                    dve.wait_ge(s_act, r_a)
                    dve.tensor_mul(c_st[:], sf[:], c_st[:]).then_inc(s_dve, 1)
                    dve.wait_ge(s_dve, r_s)
                    dve.reg_add(r_s, r_s, 5)
                    dve.tensor_add(c_st[:], c_st[:], t1[:]).then_inc(s_dve, 1)
                    dve.reg_add(r_a, r_a, 2)
                    dve.wait_ge(s_act, r_a)
                    dve.reg_add(r_a, r_a, 2)
                    dve.wait_ge(s_pe_tr, r_tw)
                    dve.reg_add(r_tw, r_tw, 1)
                    dve.tensor_mul(h_flat[par][:], so[:], tc[:]).then_inc(s_dve, 1)
                    dve.wait_ge(s_pe_tr, r_tr)
                    dve.reg_add(r_tr, r_tr, 1)
                    dve.tensor_copy(
                        hs_sb[:, bass.ds(jv * 8 + 4 + 2 * k, 2)], psum_h[par][:]
                    ).then_inc(s_dve, 1)

    for cm in reversed(ctx_list):
        cm.__exit__(None, None, None)
    nc.finalize()
    return nc


def _prep_inputs(seq, W_ih, W_hh, b_ih, b_hh):
    T = len(seq) + 2
    pad_t = T + 8
    seq = np.asarray(seq, np.float32)
    W_ih = np.asarray(W_ih, np.float32)
    W_hh = np.asarray(W_hh, np.float32)
    b_ih = np.asarray(b_ih, np.float32)
    b_hh = np.asarray(b_hh, np.float32)

    special = np.array([[0.0, 0.0, 1.0]], np.float32)
    xs = np.concatenate([special, seq, special], axis=0)  # [T, 3]
    xp_full = (
        xs.astype(np.float64) @ W_ih.astype(np.float64).T
        + (b_ih.astype(np.float64) + b_hh.astype(np.float64))
    ).astype(np.float32)  # [T, 8192]

    # W_hh viewed [gate(4), core(8), n(256), d(8), cc(2), kappa(128)]
    Wv = W_hh.reshape(4, 8, 256, 8, 2, 128)
    ones = np.zeros((128, 17), np.float32)
    ones[:, 0] = 1.0

    in_maps = []
    for r in range(NCORES):
        w_parts = []
        for jc in range(4):
            blk = Wv[GATE_OF_CHUNK[jc], r]  # [256 n, 8 d, 2 cc, 128 kappa]
            w_parts.append(
                np.ascontiguousarray(
                    blk.transpose(3, 1, 2, 0).reshape(128, 16 * 256)
                )
            )
        w_moving = np.concatenate(w_parts, axis=1)  # [128, 16384]

        xp_core = np.zeros((4, pad_t * 256), np.float16)
        for jc in range(4):
            g = GATE_OF_CHUNK[jc]
            xp_core[jc, : T * 256] = xp_full[
                :, g * 2048 + r * 256 : g * 2048 + (r + 1) * 256
            ].reshape(-1)

        in_maps.append({"w_in": w_moving, "xp_in": xp_core, "ones_in": ones})
    return in_maps


def _assemble(results, T=T_TOTAL):
    full = np.zeros((T, H), np.float32)
    for r in range(NCORES):
        hs_r = results[r]["hs_out"]  # [128, 2T] f32
        a = hs_r.reshape(128, T, 2).transpose(1, 2, 0).reshape(T, 256)
        full[:, r * 256 : (r + 1) * 256] = a
    hs = full[1:]
    h = full[-1]
    return h.copy(), hs.copy()


def kernel(seq, W_ih, W_hh, b_ih, b_hh):
    from concourse.bass_utils import run_bass_kernel_spmd

    if "nc" not in _CACHED:
        _CACHED["nc"] = _build_nc()
    nc = _CACHED["nc"]
    in_maps = _prep_inputs(seq, W_ih, W_hh, b_ih, b_hh)
    res = run_bass_kernel_spmd(nc, in_maps, core_ids=list(range(NCORES)))
    return _assemble(res.results)


# revision 14
# speedup vs baseline: 1.8386x; 1.5277x over previous
"""Self-contained Trainium2 Bass kernel for nn_Encoder (batch-1 LSTM encoder).

Reference computation (H=2048, N=4096):
  xs = [special] + seq + [special]          # [4098, 3]
  x_proj = xs @ W_ih.T + (b_ih + b_hh)      # [4098, 8192]
  scan LSTM cell over 4098 steps; return (h_final, hs[1:])

Strategy: tensor-parallel over the 4H gate dim across 8 NeuronCores.
Each core owns a 256-wide slice of h (and the 4x256 gate rows feeding it).
Per step, per core:
  PE:   4 gate chunks (order i,g,f,o), each = 1 xp-init matmul (K=1) +
        16 k-matmuls (fp32r, W moving [128,256], h k-tile stationary [128,1])
        accumulating into psum [1,256] (one bank per chunk, parity-doubled).
  ACT:  sigmoid/tanh on psum chunks (overlapped under the matvec).
  DVE:  cell update; h = sig_o * tanh(c); copy h -> hs buffer [128,2].
  PE:   2 fp32 K=1 matmuls reshape h [1,256] -> [128,2] (psum).
  Q7:   remote_dma_broadcast of the [128,2] h-slice to all 8 cores
        (register column offset = 2*core_id), double-buffered by step parity.
x_proj is precomputed on host (0.15% of FLOPs) and streamed from DRAM.
"""

import numpy as np

H = 2048
NSEQ = 4096
T_TOTAL = NSEQ + 2  # 4098
NCORES = 8
NBODY = (T_TOTAL - 2) // 4  # 1024 loop bodies of 4 steps
PAD_T = T_TOTAL + 8
GATE_OF_CHUNK = [0, 2, 1, 3]  # chunk order i,g,f,o -> torch row-block index

_CACHED = {}


def _build_nc(T=T_TOTAL, bcast=True, mini=0):
    import concourse.bass as bass
    import concourse.mybir as mybir
    from concourse.bacc import Bacc

    F32 = mybir.dt.float32
    F32R = mybir.dt.float32r
    F16 = mybir.dt.float16
    ACTF = mybir.ActivationFunctionType

    nbody = (T - 2) // 4
    pad_t = T + 8
    nc = Bacc(num_devices=NCORES)

    w_in = nc.declare_dram_parameter("w_in", [128, 16384], F16, isOutput=False)
    xp_in = nc.declare_dram_parameter("xp_in", [4, pad_t * 256], F16, isOutput=False)
    ones_in = nc.declare_dram_parameter("ones_in", [128, 17], F32R, isOutput=False)
    hs_out = nc.declare_dram_parameter("hs_out", [128, 2 * T], F16, isOutput=True)

    ctx_list = []

    class _N:
        pass

    n = _N()

    def sb(name, shape, dt_):
        cm = nc.sbuf_tensor(name, shape, dt_)
        h = cm.__enter__()
        ctx_list.append(cm)
        return h

    def psumt(name, shape, dt_):
        cm = nc.psum_tensor(name, shape, dt_)
        h = cm.__enter__()
        ctx_list.append(cm)
        return h

    w_sb = sb("w_sb", [128, 16384], F32R)
    w16_sb = sb("w16_sb", [128, 16384], F16)
    hs_sb = sb("hs_sb", [128, 2 * T], F32R)
    hs16_sb = sb("hs16_sb", [128, 2 * T], F16)
    h_buf = [sb("h_buf0", [128, 16], F32R), sb("h_buf1", [128, 16], F32R)]
    xp_pro = sb("xp_pro", [128, 512], F16)
    xp_buf = [sb("xp_b0", [128, 512], F16), sb("xp_b1", [128, 512], F16)]
    onesr_sb = sb("onesr", [128, 1], F32R)
    ones_sb = sb("ones", [128, 1], F32)
    ones16 = sb("ones16", [128, 1], F16)
    sig_i = sb("sig_i", [1, 256], F32)
    tg = sb("tg", [1, 256], F32)
    sf = sb("sf", [1, 256], F32)
    so = sb("so", [1, 256], F32)
    tc = sb("tc", [1, 256], F32)
    t1 = sb("t1", [1, 256], F32)
    c_st = sb("c_st", [1, 256], F32)
    h_flat = [sb("h_flat0", [1, 256], F32), sb("h_flat1", [1, 256], F32)]

    ps = [psumt(f"ps{b}", [128, 512], F32) for b in range(8)]
    psum_h = [ps[0][:, 256:258], ps[4][:, 256:258]]

    s_dma = nc.alloc_semaphore("s_dma")
    s_init = nc.alloc_semaphore("s_init")
    s_xp = nc.alloc_semaphore("s_xp")
    s_pe_chunk = nc.alloc_semaphore("s_pe_chunk")  # +4/step
    s_pe_tr = nc.alloc_semaphore("s_pe_tr")  # +1/step
    s_act = nc.alloc_semaphore("s_act")  # +5/step
    s_dve = nc.alloc_semaphore("s_dve")  # +5/step
    s_r = [nc.alloc_semaphore("s_r0"), nc.alloc_semaphore("s_r1")]  # arrivals
    s_l = [nc.alloc_semaphore("s_l0"), nc.alloc_semaphore("s_l1")]  # local send
    s_prep = nc.alloc_semaphore("s_prep")

    with nc.Block() as block:

        # ---------------- SYNC: loads + xp streaming + final store -----------
        @block.sync
        def _(sync):
            sync.dma_start(out=w16_sb[:], in_=w_in[:]).then_inc(s_dma, 16)
            with nc.allow_non_contiguous_dma("one-time 128x4B const load"):
                sync.dma_start(out=onesr_sb[:], in_=ones_in[:, 0:1]).then_inc(s_dma, 16)
            sync.dma_start(out=h_buf[0][:], in_=ones_in[:, 1:17]).then_inc(s_dma, 16)
            # xp prologue: steps 0,1 -> xp_pro; steps 2,3 -> buf0; 4,5 -> buf1
            sync.dma_start(
                out=xp_pro[0:128:32, 0:512], in_=xp_in[0:4, 0:512]
            ).then_inc(s_dma, 16)
            sync.dma_start(
                out=xp_buf[0][0:128:32, 0:512], in_=xp_in[0:4, 512:1024]
            ).then_inc(s_dma, 16)
            sync.dma_start(
                out=xp_buf[1][0:128:32, 0:512], in_=xp_in[0:4, 1024:1536]
            ).then_inc(s_dma, 16)
            if not (mini & 1):
                r_cons = sync.alloc_register("r_cons")
                sync.reg_mov(r_cons, 24)
                with sync.Fori(0, nbody) as jv:
                    # body j loads xp for body j+1; guard: PE consumed the
                    # buffers being overwritten
                    sync.wait_ge(s_pe_chunk, r_cons)
                    sync.reg_add(r_cons, r_cons, 16)
                    base = jv * 1024 + 1536
                    sync.dma_start(
                        out=xp_buf[0][0:128:32, 0:512],
                        in_=xp_in[0:4, bass.ds(base, 512)],
                    ).then_inc(s_xp, 16)
                    sync.dma_start(
                        out=xp_buf[1][0:128:32, 0:512],
                        in_=xp_in[0:4, bass.ds(base + 512, 512)],
                    ).then_inc(s_xp, 16)
            # final: all steps done -> convert + store hs fp16
            sync.wait_ge(s_dve, 5 * T + 1)  # incl. final fp16 convert
            sync.dma_start(out=hs_out[:], in_=hs16_sb[:]).then_inc(s_dma, 16)
            if bcast:
                sync.wait_ge(s_r[0], 16 * (T // 2))
                sync.wait_ge(s_r[1], 16 * (T // 2))
                sync.wait_ge(s_l[0], 16 * (T // 2))
                sync.wait_ge(s_l[1], 16 * (T // 2))
            sync.wait_ge(s_dma, 112)

        # ---------------- GPSIMD: memsets + broadcast loop -------------------
        @block.gpsimd
        def _(gp):
            gp.memset(c_st[:], 0.0).then_inc(s_init, 1)  # s_init: 2 total
            gp.memset(ones_sb[:], 1.0).then_inc(s_init, 1)
            gp.memset(ones16[:], 1.0).then_inc(s_init, 1)  # 3 total
            if not bcast:
                return
            pid = gp.partition_id()
            mycol = pid * 2

            r_dve = gp.alloc_register("r_dve")
            gp.reg_mov(r_dve, 5)

            def emit_bcast(t_step, col):
                # broadcast hs[:, 2t:2t+2] into h_buf[(t+1)&1] col 2*pid
                dstb = h_buf[(t_step + 1) & 1]
                gp.remote_dma_broadcast(
                    out_ap=dstb[:, bass.ds(mycol, 2)],
                    in_ap=hs_sb[:, col],
                    remote_sem=s_r[(t_step + 1) & 1],
                    local_sem=s_l[t_step & 1],
                    rdests=[(0, k) for k in range(NCORES)],
                ).then_inc(s_prep, 1)

            # prologue steps 0,1
            for t in (0, 1):
                emit_bcast(t, slice(2 * t, 2 * t + 2))
            gp.wait_ge(s_prep, 2)
            gp.wait_ge(s_dve, 5)  # copy of step 0 done
            gp.trigger_dma(1)
            gp.wait_ge(s_dve, 10)
            gp.trigger_dma(1)

            r_bp0 = gp.alloc_register("r_bp0")
            r_bp1 = gp.alloc_register("r_bp1")
            r_prep = gp.alloc_register("r_prep")
            gp.reg_mov(r_bp0, 0)
            gp.reg_mov(r_bp1, 0)
            gp.reg_mov(r_prep, 3)
            gp.reg_mov(r_dve, 15)
            with gp.Fori(0, nbody) as jv:
                for k in range(4):
                    t_par = (2 + k) & 1  # parity of t = 4j+2+k
                    emit_bcast(2 + k, bass.ds(jv * 8 + 4 + 2 * k, 2))
                    gp.wait_ge(s_prep, r_prep)
                    gp.reg_add(r_prep, r_prep, 1)
                    gp.wait_ge(s_dve, r_dve)
                    gp.reg_add(r_dve, r_dve, 5)
                    gp.trigger_dma(1)

        # ---------------- PE: matvec + reshape matmuls -----------------------
        @block.tensor
        def _(pe):
            def xp_mms(src_ap_cols, par):
                for jc in range(4):
                    pe.matmul(
                        ps[4 * par + jc][0:1, 0:256],
                        lhsT=ones16[32 * jc : 32 * jc + 1, 0:1],
                        rhs=src_ap_cols(jc),
                        start=True,
                        stop=False,
                        tile_position=(32 * jc, 0),
                    )

            def k_mms(par):
                for jc in range(4):
                    for tau in range(16):
                        ins = pe.matmul(
                            ps[4 * par + jc][0:1, 0:256],
                            lhsT=h_buf[par][:, tau : tau + 1],
                            rhs=w_sb[
                                :, (jc * 16 + tau) * 256 : (jc * 16 + tau + 1) * 256
                            ],
                            start=False,
                            stop=(tau == 15),
                        )
                        if tau == 15:
                            ins.then_inc(s_pe_chunk, 1)

            def reshape_mms(t_par):
                hf = h_flat[t_par]
                pe.matmul(
                    psum_h[t_par][:, 0:1],
                    lhsT=hf[0:1, 0:128],
                    rhs=ones_sb[0:1, 0:1],
                    start=True,
                    stop=True,
                )
                pe.matmul(
                    psum_h[t_par][:, 1:2],
                    lhsT=hf[0:1, 128:256],
                    rhs=ones_sb[0:1, 0:1],
                    start=True,
                    stop=True,
                ).then_inc(s_pe_tr, 1)

            # prologue
            pe.wait_ge(s_dma, 96)
            pe.wait_ge(s_init, 4)
            # t=0: h_buf0 zeros
            xp_mms(lambda jc: xp_pro[32 * jc : 32 * jc + 1, 0:256], 0)
            k_mms(0)
            pe.wait_ge(s_dve, 4)
            reshape_mms(0)
            # t=1
            xp_mms(lambda jc: xp_pro[32 * jc : 32 * jc + 1, 256:512], 1)
            if bcast:
                pe.wait_ge(s_r[1], 16)
            k_mms(1)
            pe.wait_ge(s_dve, 9)
            reshape_mms(1)

            r_ps = [pe.alloc_register(f"r_ps{p}") for p in range(2)]
            r_ra = [pe.alloc_register(f"r_ra{p}") for p in range(2)]
            r_hf = [pe.alloc_register(f"r_hf{p}") for p in range(2)]
            r_xp = pe.alloc_register("r_xp")
            pe.reg_mov(r_ps[0], 5)  # 5t-5 at t=2
            pe.reg_mov(r_ps[1], 10)  # t=3
            pe.reg_mov(r_ra[0], 16)  # t=2
            pe.reg_mov(r_ra[1], 32)  # t=3
            pe.reg_mov(r_hf[0], 14)  # 5t+4 at t=2
            pe.reg_mov(r_hf[1], 19)  # t=3
            pe.reg_mov(r_xp, 0)
            with pe.Fori(0, nbody) as jv:
                if not (mini & 1):
                    pe.wait_ge(s_xp, r_xp)
                    pe.reg_add(r_xp, r_xp, 32)
                for k in range(4):
                    par = k & 1
                    buf = xp_buf[k >> 1]
                    cols = (slice(0, 256), slice(256, 512))[k & 1]
                    pe.wait_ge(s_dve, r_ps[par])
                    pe.reg_add(r_ps[par], r_ps[par], 10)
                    xp_mms(lambda jc: buf[32 * jc : 32 * jc + 1, cols], par)
                    if bcast:
                        pe.wait_ge(s_r[par], r_ra[par])
                        pe.reg_add(r_ra[par], r_ra[par], 16)
                    k_mms(par)
                    pe.wait_ge(s_dve, r_hf[par])
                    pe.reg_add(r_hf[par], r_hf[par], 10)
                    reshape_mms(par)

        # ---------------- ACT ------------------------------------------------
        @block.scalar
        def _(act):
            def chain_acts(par, w_pe, w_dve_start, w_dve_tc):
                if w_dve_start is not None:
                    act.wait_ge(s_dve, w_dve_start)
                bank = ps[4 * par : 4 * par + 4]
                for idx, (dst, fn) in enumerate(
                    [
                        (sig_i, ACTF.Sigmoid),
                        (tg, ACTF.Tanh),
                        (sf, ACTF.Sigmoid),
                        (so, ACTF.Sigmoid),
                    ]
                ):
                    act.wait_ge(s_pe_chunk, w_pe(idx))
                    src = bank[[0, 1, 2, 3][idx]][0:1, 0:256]
                    act.activation(dst[:], src, fn).then_inc(s_act, 1)
                act.wait_ge(s_dve, w_dve_tc)
                act.activation(tc[:], c_st[:], ACTF.Tanh).then_inc(s_act, 1)

            # prologue (immediates)
            chain_acts(0, lambda i: 1 + i, None, 3)
            chain_acts(1, lambda i: 5 + i, 5, 8)

            r_pe = act.alloc_register("r_pe")
            r_ds = act.alloc_register("r_ds")
            r_tc = act.alloc_register("r_tc")
            act.reg_mov(r_pe, 9)
            act.reg_mov(r_ds, 10)  # 5t at t=2
            act.reg_mov(r_tc, 13)  # 5t+3 at t=2
            with act.Fori(0, nbody) as jv:
                for k in range(4):
                    par = k & 1
                    act.wait_ge(s_dve, r_ds)
                    act.reg_add(r_ds, r_ds, 5)
                    bank = ps[4 * par : 4 * par + 4]
                    for idx, (dst, fn) in enumerate(
                        [
                            (sig_i, ACTF.Sigmoid),
                            (tg, ACTF.Tanh),
                            (sf, ACTF.Sigmoid),
                            (so, ACTF.Sigmoid),
                        ]
                    ):
                        act.wait_ge(s_pe_chunk, r_pe)
                        act.reg_add(r_pe, r_pe, 1)
                        act.activation(dst[:], bank[idx][0:1, 0:256], fn).then_inc(
                            s_act, 1
                        )
                    act.wait_ge(s_dve, r_tc)
                    act.reg_add(r_tc, r_tc, 5)
                    act.activation(tc[:], c_st[:], ACTF.Tanh).then_inc(s_act, 1)

        # ---------------- DVE ------------------------------------------------
        @block.vector
        def _(dve):
            dve.wait_ge(s_dma, 96)  # all initial DMAs done
            dve.tensor_copy(w_sb[:], w16_sb[:]).then_inc(s_init, 1)  # 4 total

            def chain_dve(t_par, hs_col, w_act, w_self, w_trwar, w_tr):
                dve.wait_ge(s_act, w_act[0])
                dve.tensor_mul(t1[:], sig_i[:], tg[:]).then_inc(s_dve, 1)
                dve.wait_ge(s_act, w_act[1])
                dve.tensor_mul(c_st[:], sf[:], c_st[:]).then_inc(s_dve, 1)
                dve.wait_ge(s_dve, w_self)
                dve.tensor_add(c_st[:], c_st[:], t1[:]).then_inc(s_dve, 1)
                dve.wait_ge(s_act, w_act[2])
                if w_trwar is not None:
                    dve.wait_ge(s_pe_tr, w_trwar)
                dve.tensor_mul(h_flat[t_par][:], so[:], tc[:]).then_inc(s_dve, 1)
                dve.wait_ge(s_pe_tr, w_tr)
                dve.tensor_copy(hs_sb[:, hs_col], psum_h[t_par][:]).then_inc(
                    s_dve, 1
                )

            chain_dve(0, slice(0, 2), (2, 3, 5), 2, None, 1)
            chain_dve(1, slice(2, 4), (7, 8, 10), 7, None, 2)

            r_a = dve.alloc_register("r_a")
            r_s = dve.alloc_register("r_s")
            r_tw = dve.alloc_register("r_tw")
            r_tr = dve.alloc_register("r_tr")
            dve.reg_mov(r_a, 12)  # 5t+2 at t=2
            dve.reg_mov(r_s, 12)  # own 5t+2
            dve.reg_mov(r_tw, 1)  # t-1 at t=2
            dve.reg_mov(r_tr, 3)  # t+1 at t=2
            with dve.Fori(0, nbody) as jv:
                for k in range(4):
                    par = k & 1
                    dve.wait_ge(s_act, r_a)
                    dve.tensor_mul(t1[:], sig_i[:], tg[:]).then_inc(s_dve, 1)
                    dve.reg_add(r_a, r_a, 1)
                    dve.wait_ge(s_act, r_a)
                    dve.tensor_mul(c_st[:], sf[:], c_st[:]).then_inc(s_dve, 1)
                    dve.wait_ge(s_dve, r_s)
                    dve.reg_add(r_s, r_s, 5)
                    dve.tensor_add(c_st[:], c_st[:], t1[:]).then_inc(s_dve, 1)
                    dve.reg_add(r_a, r_a, 2)
                    dve.wait_ge(s_act, r_a)
                    dve.reg_add(r_a, r_a, 2)
                    dve.wait_ge(s_pe_tr, r_tw)
                    dve.reg_add(r_tw, r_tw, 1)
                    dve.tensor_mul(h_flat[par][:], so[:], tc[:]).then_inc(s_dve, 1)
                    dve.wait_ge(s_pe_tr, r_tr)
                    dve.reg_add(r_tr, r_tr, 1)
                    dve.tensor_copy(
                        hs_sb[:, bass.ds(jv * 8 + 4 + 2 * k, 2)], psum_h[par][:]
                    ).then_inc(s_dve, 1)
            dve.wait_ge(s_dve, 5 * T)  # own last hs write retired
            dve.tensor_copy(hs16_sb[:], hs_sb[:].bitcast(F32)).then_inc(s_dve, 1)

    for cm in reversed(ctx_list):
        cm.__exit__(None, None, None)
    nc.finalize()
    return nc


def _prep_inputs(seq, W_ih, W_hh, b_ih, b_hh):
    T = len(seq) + 2
    pad_t = T + 8
    seq = np.asarray(seq, np.float32)
    W_ih = np.asarray(W_ih, np.float32)
    W_hh = np.asarray(W_hh, np.float32)
    b_ih = np.asarray(b_ih, np.float32)
    b_hh = np.asarray(b_hh, np.float32)

    special = np.array([[0.0, 0.0, 1.0]], np.float32)
    xs = np.concatenate([special, seq, special], axis=0)  # [T, 3]
    xp_full = (
        xs.astype(np.float64) @ W_ih.astype(np.float64).T
        + (b_ih.astype(np.float64) + b_hh.astype(np.float64))
    ).astype(np.float32)  # [T, 8192]

    # W_hh viewed [gate(4), core(8), n(256), d(8), cc(2), kappa(128)]
    Wv = W_hh.reshape(4, 8, 256, 8, 2, 128)
    ones = np.zeros((128, 17), np.float32)
    ones[:, 0] = 1.0

    in_maps = []
    for r in range(NCORES):
        w_parts = []
        for jc in range(4):
            blk = Wv[GATE_OF_CHUNK[jc], r]  # [256 n, 8 d, 2 cc, 128 kappa]
            w_parts.append(
                np.ascontiguousarray(
                    blk.transpose(3, 1, 2, 0).reshape(128, 16 * 256)
                )
            )
        w_moving = np.concatenate(w_parts, axis=1).astype(np.float16)

        xp_core = np.zeros((4, pad_t * 256), np.float16)
        for jc in range(4):
            g = GATE_OF_CHUNK[jc]
            xp_core[jc, : T * 256] = xp_full[
                :, g * 2048 + r * 256 : g * 2048 + (r + 1) * 256
            ].reshape(-1)

        in_maps.append({"w_in": w_moving, "xp_in": xp_core, "ones_in": ones})
    return in_maps


def _assemble(results, T=T_TOTAL):
    full = np.zeros((T, H), np.float32)
    for r in range(NCORES):
        hs_r = np.asarray(results[r]["hs_out"], np.float32)  # [128, 2T]
        a = hs_r.reshape(128, T, 2).transpose(1, 2, 0).reshape(T, 256)
        full[:, r * 256 : (r + 1) * 256] = a
    hs = full[1:]
    h = full[-1]
    return h.copy(), hs.copy()


def kernel(seq, W_ih, W_hh, b_ih, b_hh):
    from concourse.bass_utils import run_bass_kernel_spmd

    if "nc" not in _CACHED:
        _CACHED["nc"] = _build_nc()
    nc = _CACHED["nc"]
    in_maps = _prep_inputs(seq, W_ih, W_hh, b_ih, b_hh)
    res = run_bass_kernel_spmd(nc, in_maps, core_ids=list(range(NCORES)))
    return _assemble(res.results)
